# revision 62
# baseline (speedup 1.0000x reference)
"""AIFI block (linear attention + dwconv + FFN) on 8 TRN2 NeuronCores.

Data-parallel over batch: core i computes batch element i entirely on-core.

v3: Mc-fusion + cross-rep software pipeline. The attention output pass is
folded into the depthwise conv:
    dw = sum_tap diag(w_tap) @ (ctx^T q)_shift
       = sum_tap (ctx @ diag(w_tap))^T @ q_shift
so per rep we build 18 tiny Mc = SCALE*ctx*diag(w_tap) matrices (fp8) and
the attn+dw pipeline becomes 9 shifted DoubleRow matmuls over q8. All big
matmuls run fp8 DoubleRow (K=256/instr). bproj/dw_b are folded host-side
into B1' = bn1_b - bn1_m*gr1 + A1*(bproj + Wproj^T dw_b).

Cross-rep pipeline: each emission body b carries phase-1 of rep b
(kv/ctx/q8/Mc) interleaved with the steady FFN loop of rep b-1
(dw/proj/fc1/gelu/fc2/out), so the marginal rep cost is max(engine) not
the serial sum. PSUM: fc1 2x[P,2,512] (4 banks) + fc2 2x[P,512] (2) +
shared transient pool 2x[P,512] (2) for dw/proj/kv/q/ctx/Mc.

Scales: xq=fp8(x); W*=fp8(32W); k,v = 32(k|v+bias) fp8 (one
max-with-threshold drain per tile); q8 = 32relu(q+bq) fp8; ctx
accumulated in 4 psum groups -> f32 ctxacc -> bf16 ctxT_sb = SCALE*ctx^T;
mcT = 16*SCALE*ctx*w fp8; dw_ps = 512*dwconv -> dwc8 = 16*dwconv;
proj_ps = 512*proj -> v_sb = (A1A2/512)ps + u12; t1_8 = fp8(t1);
h8 = fp8(gelu) (scale 1/32, bias bfc1, [P,1024] nt-pair drains);
fc2_ps = 32*fc2; out = (A2/32)ps + v_sb.
"""

import sys

import numpy as np

_REPO = "/opt/trn_rl_repo"
if _REPO not in sys.path:
    sys.path.insert(0, _REPO)

B, C, HH, WW = 8, 256, 64, 64
N = HH * WW  # 4096 tokens
NH, HD = 8, 32
CM = 2048
EPS = 1e-5
SCALE = HD ** -0.5
P = 128
NTC = 512          # columns per n-tile
NT = N // NTC      # 8 n-tiles
TT = N // P        # 32 token tiles
MH = CM // P       # 16 hidden chunks
YB = NTC // WW     # 8 y-rows per n-tile
SW = 32.0          # fp8 weight pre-scale
BETA = 16.0        # Mc fp8 scale
SD = 16.0          # dwc fp8 scale

_CACHE = {}
_DEBUG = False

TAPS = [(0, 0), (0, -1), (0, 1), (-1, -1), (-1, 0), (-1, 1),
        (1, -1), (1, 0), (1, 1)]


def _build_nc(reps=1):
    import concourse.bass as bass
    import concourse.tile as tile
    from concourse import bacc, mybir
    from concourse.masks import make_identity

    f32 = mybir.dt.float32
    bf16 = mybir.dt.bfloat16
    fp8 = mybir.dt.float8e4
    Relu = mybir.ActivationFunctionType.Relu
    Gelu = mybir.ActivationFunctionType.Gelu
    Copy = mybir.ActivationFunctionType.Copy
    add = mybir.AluOpType.add
    mult = mybir.AluOpType.mult
    amax = mybir.AluOpType.max
    DR = mybir.MatmulPerfMode.DoubleRow

    nc = bacc.Bacc(None, target_bir_lowering=False)

    x_ext = nc.declare_dram_parameter("x", [C, HH, WW], bf16, isOutput=False)
    xq_ext = nc.declare_dram_parameter("xq", [2, P, N], fp8, isOutput=False)
    wqkv_ext = nc.declare_dram_parameter("wqkv8", [2, P, 3 * C], fp8,
                                         isOutput=False)
    wproj_ext = nc.declare_dram_parameter("wproj8", [2, P, C], fp8,
                                          isOutput=False)
    wfc1_ext = nc.declare_dram_parameter("wfc18", [2, P, CM], fp8,
                                         isOutput=False)
    wfc2_ext = nc.declare_dram_parameter("wfc28", [MH, P, C], fp8,
                                         isOutput=False)
    dww_ext = nc.declare_dram_parameter("dww", [C, 9], f32, isOutput=False)
    # pcst columns: 0=32*bq 1=A1A2/512 2=u12b 3=A2/32 4=invA2 5=negB2oA2
    pcst_ext = nc.declare_dram_parameter("pcst", [C, 8], f32, isOutput=False)
    bkv_ext = nc.declare_dram_parameter("bkv8", [2, 2 * C], fp8,
                                        isOutput=False)
    bfc1_ext = nc.declare_dram_parameter("bfc1c", [P, MH], f32, isOutput=False)
    out_ext = nc.declare_dram_parameter("out", [C, HH, WW], f32, isOutput=True)
    dbg = {}
    if _DEBUG:
        for nm, shape, dt in [
            ("dbg_q8", [P, 2, HH * (WW + 2) + 2], fp8),
            ("dbg_kv0", [P, TT // 2, 2 * C], fp8),
            ("dbg_kv1", [P, TT // 2, 2 * C], fp8),
            ("dbg_ctxacc", [P, 2 * P], f32),
            ("dbg_g0ps", [P, 2 * P], f32),
            ("dbg_dwps", [P, 4 * (WW + 2)], f32),
            ("dbg_ctxT", [P, 2 * P], mybir.dt.bfloat16),
            ("dbg_mcT", [P, 2, 9, 2 * P], fp8),
            ("dbg_dwc8", [P, 2, N], fp8),
            ("dbg_vsb", [P, N], mybir.dt.bfloat16),
            ("dbg_t18", [P, 2, N], fp8),
        ]:
            dbg[nm] = nc.declare_dram_parameter(nm, shape, dt, isOutput=True)

    with tile.TileContext(nc) as tc:
        with (
            tc.tile_pool(name="persist", bufs=1) as persist,
            tc.tile_pool(name="small", bufs=1) as small,
            tc.tile_pool(name="xbpool", bufs=2) as xbpool,
            tc.tile_pool(name="dtpool", bufs=2) as dtpool,
            tc.tile_pool(name="kvpool", bufs=2) as kvpool,
            tc.tile_pool(name="upool", bufs=1) as upool,
            tc.tile_pool(name="hpool", bufs=2) as hpool,
            tc.tile_pool(name="outsb", bufs=2) as outsb_pool,
            tc.tile_pool(name="psFc1", bufs=2, space="PSUM") as psFc1,
            tc.tile_pool(name="psHold", bufs=2, space="PSUM") as psHold,
            tc.tile_pool(name="psX", bufs=2, space="PSUM") as psX,
        ):
            # ---- constants built once --------------------------------
            ident = small.tile([P, P], bf16, tag="ident", name="ident")
            make_identity(nc, ident[:])
            ones8 = small.tile([1, 2, P], fp8, tag="ones8", name="ones8")
            nc.vector.memset(ones8[:], 1.0)
            thr = small.tile([P, 2 * C], bf16, tag="thr", name="thr")
            nc.vector.memset(thr[:, 0:C], 0.0)
            nc.vector.memset(thr[:, C:2 * C], -1e30)
            # double-buffered Mc (gen = rep % 2); zero halves stay zero
            mcT = [persist.tile([P, 2, 9, 2 * P], fp8, tag=f"mcT{g}",
                                name="mcT") for g in range(2)]
            for g in range(2):
                nc.vector.memset(mcT[g][:], 0.0)
            # q8 with 66-col padded rows (zero pads at col 0 and 65) so dw
            # taps are flat 1-D offset slices; memset once, drains only
            # touch data columns
            WP = WW + 2
            q8g = [persist.tile([P, 2, HH * WP + 2], fp8, tag=f"q8_{g}",
                                name="q8") for g in range(2)]
            for g in range(2):
                nc.gpsimd.memset(q8g[g][:], 0.0)
            ctxacc = [small.tile([P, 2 * P], f32, tag=f"ctxacc{g}",
                                 name="ctxacc") for g in range(2)]
            # block-diagonal at 32x32 head level: zero once, only diagonal
            # head blocks rewritten per rep
            ctxT_sb = [small.tile([P, 2 * P], bf16, tag=f"ctxT{g}",
                                  name="ctxT") for g in range(2)]
            for g in range(2):
                nc.vector.memset(ctxT_sb[g][:], 0.0)

            pcst = [small.tile([P, 8], f32, tag=f"pcst_{m}", name="pcst")
                    for m in range(2)]
            bq32 = [pcst[m][:, 0:1] for m in range(2)]
            A1A2s = [pcst[m][:, 1:2] for m in range(2)]
            u12b = [pcst[m][:, 2:3] for m in range(2)]
            A2_32 = [pcst[m][:, 3:4] for m in range(2)]
            invA2 = [pcst[m][:, 4:5] for m in range(2)]
            negB2oA2 = [pcst[m][:, 5:6] for m in range(2)]
            A1A2f = [pcst[m][:, 6:7] for m in range(2)]
            bkv_sb = small.tile([1, 2, 2 * C], fp8, tag="bkv", name="bkv")
            bfc1_sb = small.tile([P, MH], f32, tag="bfc1_sb", name="bfc1_sb")
            dwwt = [small.tile([P, 9], f32, tag=f"dww_{m}", name="dwwt")
                    for m in range(2)]
            diag_c = [small.tile([P, 9, P], bf16, tag=f"diagc_{m}",
                                 name="diagc") for m in range(2)]
            wqkv = persist.tile([P, 2, 3 * C], fp8, tag="wqkv", name="wqkv")
            wproj = persist.tile([P, 2, C], fp8, tag="wproj", name="wproj")
            wfc1 = persist.tile([P, 2, CM], fp8, tag="wfc1", name="wfc1")
            wfc2 = persist.tile([P, MH, C], fp8, tag="wfc2", name="wfc2")
            xq = persist.tile([P, 2, N], fp8, tag="xq", name="xq")
            # u12 as per-chunk tiles so early consumers don't wait on the
            # last chunk's write (coarse tile-level dependencies)
            u12 = [[[upool.tile([P, N // 4], bf16, tag=f"u12_{m}_{g}_{j}",
                                name="u12")
                     for j in range(4)] for m in range(2)] for g in range(2)]
            v_sb = [upool.tile([P, N], bf16, tag=f"vsb_{m}", name="v_sb")
                    for m in range(2)]

            x2d = x_ext[:].rearrange("c h w -> c (h w)")
            out2d = out_ext[:].rearrange("c h w -> c (h w)")

            # ---- one-time weight/constant loads --------------------------
            dma_engs = [nc.sync, nc.scalar, nc.sync, nc.scalar]
            for m in range(2):
                nc.gpsimd.dma_start(
                    out=pcst[m][:], in_=pcst_ext[m * P:(m + 1) * P, :]
                )
            nc.gpsimd.dma_start(out=bkv_sb[:], in_=bkv_ext[:].rearrange(
                "k n -> (k n)").rearrange("(o k n) -> o k n", o=1, k=2))
            nc.gpsimd.dma_start(out=bfc1_sb[:], in_=bfc1_ext[:])
            nc.sync.dma_start(
                out=wqkv[:], in_=wqkv_ext[:].rearrange("k p f -> p k f")
            )
            nc.gpsimd.dma_start(
                out=wproj[:], in_=wproj_ext[:].rearrange("k p c -> p k c")
            )
            nc.scalar.dma_start(
                out=wfc1[:], in_=wfc1_ext[:].rearrange("k p f -> p k f")
            )
            for k in range(4):
                dma_engs[k % 2].dma_start(
                    out=wfc2[:, 4 * k:4 * (k + 1), :],
                    in_=wfc2_ext[:].rearrange("k p c -> p k c")[
                        :, 4 * k:4 * (k + 1), :
                    ],
                )
            for m in range(2):
                nc.gpsimd.dma_start(
                    out=dwwt[m][:], in_=dww_ext[m * P:(m + 1) * P, :]
                )
                for i in range(9):
                    nc.gpsimd.tensor_scalar_mul(
                        diag_c[m][:, i, :], ident[:], dwwt[m][:, i:i + 1]
                    )

            def emit_loads(r):
                st = {}
                st["xb"] = [xbpool.tile([P, N], bf16, tag="xbpool", name="xb")
                            for _ in range(2)]
                xq_r = xq_ext[:].rearrange("k p n -> p k n")
                for j in range(8):
                    s = slice(j * 512, (j + 1) * 512)
                    dma_engs[j % 4].dma_start(out=xq[:, :, s],
                                              in_=xq_r[:, :, s])
                for j in range(4):
                    s = slice(j * 1024, (j + 1) * 1024)
                    for m in range(2):
                        rows = slice(m * P, (m + 1) * P)
                        dma_engs[(2 * j + m) % 4].dma_start(
                            out=st["xb"][m][:, s], in_=x2d[rows, s]
                        )
                st["kv8"] = [kvpool.tile([P, TT // 2, 2 * C], fp8,
                                         tag="kvpool", name="kv8")
                             for _ in range(2)]
                st["q8"] = q8g[r % 2]
                return st

            # ---------------- phase-1 pieces (rep r) ----------------------
            def kv_step(st, tt):
                ti, j = divmod(tt, TT // 2)
                tcols = slice(tt * P, (tt + 1) * P)
                ps = psX.tile([P, 2 * C], f32, tag="psX", name="kv_ps")
                nc.tensor.matmul(
                    ps[:], xq[:, :, tcols], wqkv[:, :, C:3 * C],
                    start=True, stop=False, perf_mode=DR,
                    skip_group_check=True,
                )
                nc.tensor.matmul(
                    ps[:], ones8[:], bkv_sb[:],
                    start=False, stop=True, perf_mode=DR,
                    skip_group_check=True,
                )
                # k = relu(32(k+bk)), v = 32(v+bv): max with {0 | -big}
                nc.vector.scalar_tensor_tensor(
                    st["kv8"][ti][:, j, :], ps[:], 1.0, thr[:], mult, amax,
                )

            def q_step(st, nt):
                cols = slice(nt * NTC, (nt + 1) * NTC)
                for m in range(2):
                    ps = psX.tile([P, NTC], f32, tag="psX", name="q_ps")
                    nc.tensor.matmul(
                        ps[:],
                        wqkv[:, :, m * P:(m + 1) * P],
                        xq[:, :, cols],
                        start=True, stop=True, perf_mode=DR,
                    )
                    q8v = st["q8"][:, m, 1:1 + HH * WP].rearrange(
                        "p (y x) -> p y x", x=WP)
                    nc.vector.tensor_scalar(
                        q8v[:, nt * YB:(nt + 1) * YB, 1:WW + 1],
                        ps[:].rearrange("p (y x) -> p y x", x=WW),
                        bq32[m], 0.0, add, amax,
                    )

            def ctx_group(st, g, gen):
                # pairs 4g..4g+3 (token tiles 8g..8g+7) -> psum -> ctxacc
                ps = psX.tile([P, 2 * P], f32, tag="psX", name="ctxg_ps")
                for pr in range(4):
                    pair = 4 * g + pr
                    ti, j = divmod(2 * pair, TT // 2)
                    for m in range(2):
                        # one start per psum bank: start marks the whole
                        # 2KB zero-region, so only the first mm may start
                        nc.tensor.matmul(
                            ps[:, m * P:(m + 1) * P],
                            st["kv8"][ti][:, j:j + 2,
                                          C + m * P:C + (m + 1) * P],
                            st["kv8"][ti][:, j:j + 2, m * P:(m + 1) * P],
                            start=(pr == 0 and m == 0),
                            stop=(pr == 3 and m == 1),
                            perf_mode=DR,
                            skip_group_check=True,
                        )
                if g == 0:
                    nc.vector.tensor_scalar(
                        ctxacc[gen][:], ps[:], SCALE / (SW * SW), None, mult,
                    )
                    if _DEBUG and st["rep"] == 0:
                        sb = small.tile([P, 2 * P], f32, tag="dbgg0",
                                        name="dbgg0")
                        nc.vector.tensor_copy(sb[:], ps[:])
                        nc.sync.dma_start(out=dbg["dbg_g0ps"][:], in_=sb[:])
                else:
                    nc.vector.scalar_tensor_tensor(
                        ctxacc[gen][:], ps[:], SCALE / (SW * SW),
                        ctxacc[gen][:], mult, add,
                    )

            def mc_build(gen):
                # extract only the within-head 32x32 diagonal blocks
                for h in range(NH):
                    m, r = divmod(h, 4)
                    rows = slice(HD * r, HD * r + HD)
                    cols = slice(m * P + HD * r, m * P + HD * r + HD)
                    nc.vector.tensor_copy(ctxT_sb[gen][rows, cols],
                                          ctxacc[gen][rows, cols])
                for m in range(2):
                    for g0, g1 in ((0, 4), (4, 8), (8, 9)):
                        ng = g1 - g0
                        ps = psX.tile([P, ng * P], f32, tag="psX",
                                      name="mc_ps")
                        for i in range(g0, g1):
                            nc.tensor.matmul(
                                ps[:, (i - g0) * P:(i - g0 + 1) * P],
                                ctxT_sb[gen][:, m * P:(m + 1) * P],
                                diag_c[m][:, i, :],
                                start=(i == g0), stop=(i == g1 - 1),
                                skip_group_check=True,
                            )
                        nc.scalar.activation(
                            mcT[gen][:, m, g0:g1, m * P:(m + 1) * P],
                            ps[:].rearrange("p (g q) -> p g q", g=ng),
                            Copy, bias=0.0, scale=BETA,
                        )

            # ---------------- steady pieces (rep s) -----------------------
            def clip(dy, dx, y0):
                ys = max(y0, -dy)
                ye = min(y0 + YB, HH - dy)
                xs = max(0, -dx)
                xe = min(WW, WW - dx)
                return ys, ye, xs, xe

            def u12_piece(st, gen, j):
                # eager u12 (rep r) chunk j of 4, both m chunks; bf16
                # in/out SBUF on DVE hits the 2x path
                s = slice(j * 1024, (j + 1) * 1024)
                for m in range(2):
                    nc.vector.tensor_scalar(
                        u12[gen][m][j][:], st["xb"][m][:, s],
                        A1A2f[m], u12b[m], mult, add,
                    )

            def dw_phase(st, gen, nt, mo):
                # two 4-row halves; taps are flat 1-D offset slices of the
                # 66-col padded q8 (out pad cols compute junk, never read)
                q8 = st["q8"]
                for rh in range(2):
                    r0 = nt * YB + rh * 4
                    ps = psX.tile([P, 4 * WP], f32, tag="psX", name="dw_ps")
                    for i, (dy, dx) in enumerate(TAPS):
                        ysr = max(r0, -dy)
                        yer = min(r0 + 4, HH - dy)
                        ti = (dy + 1) * 3 + (dx + 1)
                        nc.tensor.matmul(
                            ps[:, (ysr - r0) * WP:(yer - r0) * WP],
                            mcT[gen][:, :, ti, mo * P:(mo + 1) * P],
                            q8[:, :, 1 + (ysr + dy) * WP + dx:
                               1 + (yer + dy) * WP + dx],
                            start=(i == 0), stop=(i == 8),
                            perf_mode=DR,
                            skip_group_check=True,
                        )
                    if _DEBUG and st["rep"] == 0 and nt == 0 and mo == 0 \
                            and rh == 0:
                        sb = small.tile([P, 4 * WP], f32, tag="dbgdw",
                                        name="dbgdw")
                        nc.vector.tensor_copy(sb[:], ps[:])
                        nc.sync.dma_start(out=dbg["dbg_dwps"][:], in_=sb[:])
                    # dwc8 = 16*dwconv = ps/32 (dw_b folded into B1')
                    nc.vector.tensor_scalar(
                        st["dwc8"][:, mo, r0 * WW:(r0 + 4) * WW].rearrange(
                            "p (y x) -> p y x", x=WW),
                        ps[:].rearrange("p (y x) -> p y x", x=WP)[
                            :, :, 1:WW + 1],
                        1.0 / SW, None, mult,
                    )

            def proj_phase(st, gen, nt, mo):
                cols = slice(nt * NTC, (nt + 1) * NTC)
                ps = psX.tile([P, NTC], f32, tag="psX", name="proj_ps")
                nc.tensor.matmul(
                    ps[:],
                    wproj[:, :, mo * P:(mo + 1) * P],
                    st["dwc8"][:, :, cols],
                    start=True, stop=True, perf_mode=DR,
                )
                nc.vector.scalar_tensor_tensor(
                    v_sb[mo][:, cols], ps[:], A1A2s[mo],
                    u12[gen][mo][nt // 2][:, (nt % 2) * NTC:
                                          (nt % 2 + 1) * NTC],
                    mult, add,
                )
                nc.gpsimd.tensor_scalar(
                    st["t1_8"][:, mo, cols], v_sb[mo][:, cols],
                    invA2[mo], negB2oA2[mo], mult, add,
                )

            def fc1_piece(st, p, k0, k1):
                n0 = 2 * p
                colsA = slice(n0 * NTC, (n0 + 1) * NTC)
                colsB = slice((n0 + 1) * NTC, (n0 + 2) * NTC)
                if k0 == 0:
                    st["h_sb"] = hpool.tile([P, MH, 2 * NTC], fp8,
                                            tag="hpool", name="h_sb")
                for kt in range(k0, k1):
                    ps = psFc1.tile([P, 2, NTC], f32, tag="psFc1",
                                    name="fc1_ps")
                    for half, cols in ((0, colsA), (1, colsB)):
                        nc.tensor.matmul(
                            ps[:, half, :],
                            wfc1[:, :, kt * P:(kt + 1) * P],
                            st["t1_8"][:, :, cols],
                            start=True, stop=True, perf_mode=DR,
                            skip_group_check=True,
                        )
                    nc.scalar.activation(
                        st["h_sb"][:, kt, :],
                        ps[:].rearrange("p h q -> p (h q)"),
                        Gelu, bias=bfc1_sb[:, kt:kt + 1], scale=1.0 / SW,
                    )

            def fc2_piece(st, p, half, ktp0, ktp1, fc2_ps):
                for ktp in range(ktp0, ktp1):
                    kt = 2 * ktp
                    for mo in range(2):
                        nc.tensor.matmul(
                            fc2_ps[mo][:],
                            wfc2[:, kt:kt + 2, mo * P:(mo + 1) * P],
                            st["h_sb"][:, kt:kt + 2,
                                       half * NTC:(half + 1) * NTC],
                            start=(ktp == 0),
                            stop=(ktp == MH // 2 - 1),
                            perf_mode=DR,
                            skip_group_check=True,
                        )

            def out_phase(nt, fc2_ps):
                cols = slice(nt * NTC, (nt + 1) * NTC)
                for mo in range(2):
                    ot = outsb_pool.tile([P, NTC], f32, tag="outsb",
                                         name="outsb")
                    nc.vector.scalar_tensor_tensor(
                        ot[:], fc2_ps[mo][:], A2_32[mo],
                        v_sb[mo][:, cols], mult, add,
                    )
                    nc.sync.dma_start(
                        out=out2d[mo * P:(mo + 1) * P, cols], in_=ot[:],
                    )

            # ---------------- piece lists + interleave --------------------
            def phase1_pieces(st, gen):
                ps_list = []
                for tt in range(TT):
                    ps_list.append(lambda tt=tt: kv_step(st, tt))
                    if tt % 4 == 3:
                        ps_list.append(lambda nt=tt // 4: q_step(st, nt))
                    if tt % 8 == 1 and tt > 1:
                        ps_list.append(
                            lambda j=(tt - 9) // 8: u12_piece(st, gen, j))
                    if tt % 8 == 7 and tt >= 15:
                        g = (tt - 15) // 8  # groups 0..2 after tiles 15,23,31
                        ps_list.append(lambda g=g: ctx_group(st, g, gen))
                ps_list.append(lambda: u12_piece(st, gen, 3))
                ps_list.append(lambda: ctx_group(st, 3, gen))
                ps_list.append(lambda: mc_build(gen))
                return ps_list

            def get_fc2(st, key):
                st["fc2_tiles"][key] = [psHold.tile([P, NTC], f32,
                                                    tag="psHold",
                                                    name="fc2_ps")
                                        for _ in range(2)]
                return st["fc2_tiles"][key]

            def steady_head_pieces(st, gen):
                # pipeline-fill of rep r's steady loop, emitted at the tail
                # of body r (overlaps rep r-1's drain)
                st["dwc8"] = dtpool.tile([P, 2, N], fp8, tag="dtpool",
                                         name="dwc8")
                st["t1_8"] = dtpool.tile([P, 2, N], fp8, tag="dtpool",
                                         name="t1_8")
                st["fc2_tiles"] = {}

                def pre(nt):
                    dw_phase(st, gen, nt, 0)
                    dw_phase(st, gen, nt, 1)
                    proj_phase(st, gen, nt, 0)
                    proj_phase(st, gen, nt, 1)

                def h1():
                    get_fc2(st, (0, 0))
                    fc1_piece(st, 0, 0, 4)

                return [lambda: pre(0), lambda: pre(1), h1]

            def steady_rest_pieces(st, gen):
                ps_list = []
                fc2_tiles = st["fc2_tiles"]
                for p in range(NT // 2):
                    def s1(p=p):
                        if p > 0:
                            get_fc2(st, (p, 0))
                            fc1_piece(st, p, 0, 4)
                    def s2(p=p):
                        if 2 * p + 2 < NT:
                            dw_phase(st, gen, 2 * p + 2, 0)
                            dw_phase(st, gen, 2 * p + 2, 1)
                    def s3(p=p):
                        fc1_piece(st, p, 4, 8)
                        fc2_piece(st, p, 0, 0, 4, fc2_tiles[(p, 0)])
                    def s4(p=p):
                        if 2 * p + 2 < NT:
                            proj_phase(st, gen, 2 * p + 2, 0)
                            proj_phase(st, gen, 2 * p + 2, 1)
                    def s5(p=p):
                        fc1_piece(st, p, 8, 12)
                        fc2_piece(st, p, 0, 4, 6, fc2_tiles[(p, 0)])
                    def s6(p=p):
                        if 2 * p + 3 < NT:
                            dw_phase(st, gen, 2 * p + 3, 0)
                            dw_phase(st, gen, 2 * p + 3, 1)
                    def s7(p=p):
                        fc1_piece(st, p, 12, 16)
                        fc2_piece(st, p, 0, 6, 8, fc2_tiles[(p, 0)])
                        out_phase(2 * p, fc2_tiles[(p, 0)])
                        get_fc2(st, (p, 1))
                        fc2_piece(st, p, 1, 0, 4, fc2_tiles[(p, 1)])
                    def s8(p=p):
                        if 2 * p + 3 < NT:
                            proj_phase(st, gen, 2 * p + 3, 0)
                            proj_phase(st, gen, 2 * p + 3, 1)
                    def s9(p=p):
                        fc2_piece(st, p, 1, 4, 8, fc2_tiles[(p, 1)])
                        out_phase(2 * p + 1, fc2_tiles[(p, 1)])
                    if p == 0:
                        ps_list += [s2, s3, s4, s5, s6, s7, s8, s9]
                    else:
                        ps_list += [s1, s2, s3, s4, s5, s6, s7, s8, s9]
                return ps_list

            def emit_debug(st, gen):
                nc.sync.dma_start(out=dbg["dbg_q8"][:], in_=st["q8"][:])
                nc.sync.dma_start(out=dbg["dbg_kv0"][:], in_=st["kv8"][0][:])
                nc.sync.dma_start(out=dbg["dbg_kv1"][:], in_=st["kv8"][1][:])
                nc.sync.dma_start(out=dbg["dbg_ctxacc"][:],
                                  in_=ctxacc[gen][:])
                nc.sync.dma_start(out=dbg["dbg_ctxT"][:], in_=ctxT_sb[gen][:])
                nc.sync.dma_start(out=dbg["dbg_mcT"][:], in_=mcT[gen][:])
                nc.sync.dma_start(out=dbg["dbg_dwc8"][:], in_=st["dwc8"][:])
                nc.sync.dma_start(out=dbg["dbg_vsb"][:], in_=v_sb[0][:])
                nc.sync.dma_start(out=dbg["dbg_t18"][:], in_=st["t1_8"][:])

            states = {}
            for body in range(reps + 1):
                p1 = []
                p2 = []
                if body < reps:
                    st = emit_loads(body)
                    st["rep"] = body
                    states[body] = st
                    p1 = (phase1_pieces(st, body % 2)
                          + steady_head_pieces(st, body % 2))
                if body >= 1:
                    sprev = states[body - 1]
                    p2 = steady_rest_pieces(sprev, (body - 1) % 2)
                # interleave with phase-1 (+ next steady head) finishing at
                # ~80% of the steady list, so the head's pipeline-fill chain
                # overlaps the previous rep's fc tail
                na, nb = len(p1), len(p2)
                ia = ib = 0
                while ia < na or ib < nb:
                    if ib < nb and (ia >= na or ia * nb * 10 >= ib * na * 6):
                        p2[ib]()
                        ib += 1
                    else:
                        p1[ia]()
                        ia += 1
                if body >= 1:
                    if _DEBUG and body == reps:
                        emit_debug(states[body - 1], (body - 1) % 2)
                    del states[body - 1]

    nc.compile()
    return nc


def _get_nc(reps=1):
    key = ("nc", reps)
    if key not in _CACHE:
        _CACHE[key] = _build_nc(reps)
    return _CACHE[key]


def _prep_shared(inputs):
    import ml_dtypes

    bf = ml_dtypes.bfloat16
    e4 = ml_dtypes.float8_e4m3
    f = lambda k: np.asarray(inputs[k], dtype=np.float32)

    rs1 = 1.0 / np.sqrt(f("bn1_v") + EPS)
    gr1 = f("bn1_g") * rs1
    A1 = gr1 + f("alpha1")
    # dw_b and bproj folded: proj_full = Wproj^T(dwconv + dw_b) + bproj
    B1 = (f("bn1_b") - f("bn1_m") * gr1
          + A1 * (f("bproj") + f("Wproj").T @ f("dw_b")))
    rs2 = 1.0 / np.sqrt(f("bn2_v") + EPS)
    gr2 = f("bn2_g") * rs2
    A2 = gr2 + f("alpha2")
    B2 = f("bn2_b") - f("bn2_m") * gr2 + A2 * f("bfc2")

    A1A2 = A1 * A2
    u12b = A2 * B1 + B2
    invA2 = 1.0 / A2
    negB2oA2 = -B2 / A2

    pcst = np.stack(
        [f("bqkv")[:C] * SW, A1A2 / (SD * SW), u12b, A2 / SW, invA2,
         negB2oA2, A1A2, np.zeros_like(A2)],
        axis=1,
    )
    bkv8 = np.zeros((2, 2 * C), np.float32)
    bkv8[0] = f("bqkv")[C:] * SW

    return {
        "wqkv8": np.ascontiguousarray(
            (f("Wqkv") * SW).astype(e4).reshape(2, P, 3 * C)),
        "wproj8": np.ascontiguousarray(
            (f("Wproj") * SW).astype(e4).reshape(2, P, C)),
        "wfc18": np.ascontiguousarray(
            (f("Wfc1") * SW).astype(e4).reshape(2, P, CM)),
        "wfc28": np.ascontiguousarray(
            (f("Wfc2") * SW).astype(e4).reshape(MH, P, C)),
        "dww": np.ascontiguousarray(f("dw_w").reshape(C, 9)),
        "pcst": np.ascontiguousarray(pcst.astype(np.float32)),
        "bkv8": np.ascontiguousarray(bkv8.astype(e4)),
        "bfc1c": np.ascontiguousarray(f("bfc1").reshape(MH, P).T),
    }


def _per_core_maps(inputs):
    import ml_dtypes

    shared = _prep_shared(inputs)
    xf = np.asarray(inputs["x"], dtype=np.float32).reshape(B, C, HH, WW)
    in_maps = []
    for i in range(B):
        x2d = xf[i].reshape(C, N)
        in_maps.append(dict(
            shared,
            x=np.ascontiguousarray(x2d.astype(ml_dtypes.bfloat16)
                                   ).reshape(C, HH, WW),
            xq=np.ascontiguousarray(
                x2d.astype(ml_dtypes.float8_e4m3).reshape(2, P, N)),
        ))
    return in_maps


def kernel(**inputs):
    from concourse.bass_utils import run_bass_kernel_spmd

    nc = _get_nc()
    in_maps = _per_core_maps(inputs)
    res = run_bass_kernel_spmd(nc, in_maps, core_ids=list(range(B)))
    return np.stack([res.results[i]["out"] for i in range(B)], axis=0)


def make_in_maps(inputs):
    return _per_core_maps(inputs)


# revision 69
# speedup vs baseline: 31.3853x; 31.3853x over previous
"""AIFI block (linear attention + dwconv + FFN) on 8 TRN2 NeuronCores.

Data-parallel over batch: core i computes batch element i entirely on-core.

v3: Mc-fusion + cross-rep software pipeline. The attention output pass is
folded into the depthwise conv:
    dw = sum_tap diag(w_tap) @ (ctx^T q)_shift
       = sum_tap (ctx @ diag(w_tap))^T @ q_shift
so per rep we build 18 tiny Mc = SCALE*ctx*diag(w_tap) matrices (fp8) and
the attn+dw pipeline becomes 9 shifted DoubleRow matmuls over q8. All big
matmuls run fp8 DoubleRow (K=256/instr). bproj/dw_b are folded host-side
into B1' = bn1_b - bn1_m*gr1 + A1*(bproj + Wproj^T dw_b).

Cross-rep pipeline: each emission body b carries phase-1 of rep b
(kv/ctx/q8/Mc) interleaved with the steady FFN loop of rep b-1
(dw/proj/fc1/gelu/fc2/out), so the marginal rep cost is max(engine) not
the serial sum. PSUM: fc1 2x[P,2,512] (4 banks) + fc2 2x[P,512] (2) +
shared transient pool 2x[P,512] (2) for dw/proj/kv/q/ctx/Mc.

Scales: xq=fp8(x); W*=fp8(32W); k,v = 32(k|v+bias) fp8 (one
max-with-threshold drain per tile); q8 = 32relu(q+bq) fp8; ctx
accumulated in 4 psum groups -> f32 ctxacc -> bf16 ctxT_sb = SCALE*ctx^T;
mcT = 16*SCALE*ctx*w fp8; dw_ps = 512*dwconv -> dwc8 = 16*dwconv;
proj_ps = 512*proj -> v_sb = (A1A2/512)ps + u12; t1_8 = fp8(t1);
h8 = fp8(gelu) (scale 1/32, bias bfc1, [P,1024] nt-pair drains);
fc2_ps = 32*fc2; out = (A2/32)ps + v_sb.
"""

import sys

import numpy as np

_REPO = "/opt/trn_rl_repo"
if _REPO not in sys.path:
    sys.path.insert(0, _REPO)

B, C, HH, WW = 8, 256, 64, 64
N = HH * WW  # 4096 tokens
NH, HD = 8, 32
CM = 2048
EPS = 1e-5
SCALE = HD ** -0.5
P = 128
NTC = 512          # columns per n-tile
NT = N // NTC      # 8 n-tiles
TT = N // P        # 32 token tiles
MH = CM // P       # 16 hidden chunks
YB = NTC // WW     # 8 y-rows per n-tile
SW = 32.0          # fp8 weight pre-scale
BETA = 16.0        # Mc fp8 scale
SD = 16.0          # dwc fp8 scale

_CACHE = {}
_DEBUG = False

TAPS = [(0, 0), (0, -1), (0, 1), (-1, -1), (-1, 0), (-1, 1),
        (1, -1), (1, 0), (1, 1)]


def _build_nc(reps=1):
    import concourse.bass as bass
    import concourse.tile as tile
    from concourse import bacc, mybir
    from concourse.masks import make_identity

    f32 = mybir.dt.float32
    bf16 = mybir.dt.bfloat16
    fp8 = mybir.dt.float8e4
    Relu = mybir.ActivationFunctionType.Relu
    Gelu = mybir.ActivationFunctionType.Gelu
    Copy = mybir.ActivationFunctionType.Copy
    add = mybir.AluOpType.add
    mult = mybir.AluOpType.mult
    amax = mybir.AluOpType.max
    DR = mybir.MatmulPerfMode.DoubleRow

    nc = bacc.Bacc(None, target_bir_lowering=False)

    x_ext = nc.declare_dram_parameter("x", [C, HH, WW], bf16, isOutput=False)
    xq_ext = nc.declare_dram_parameter("xq", [2, P, N], fp8, isOutput=False)
    wqkv_ext = nc.declare_dram_parameter("wqkv8", [2, P, 3 * C], fp8,
                                         isOutput=False)
    wproj_ext = nc.declare_dram_parameter("wproj8", [2, P, C], fp8,
                                          isOutput=False)
    wfc1_ext = nc.declare_dram_parameter("wfc18", [2, P, CM], fp8,
                                         isOutput=False)
    wfc2_ext = nc.declare_dram_parameter("wfc28", [MH, P, C], fp8,
                                         isOutput=False)
    dww_ext = nc.declare_dram_parameter("dww", [C, 9], f32, isOutput=False)
    # pcst columns: 0=32*bq 1=A1A2/512 2=u12b 3=A2/32 4=invA2 5=negB2oA2
    pcst_ext = nc.declare_dram_parameter("pcst", [C, 8], f32, isOutput=False)
    bkv_ext = nc.declare_dram_parameter("bkv8", [2, 2 * C], fp8,
                                        isOutput=False)
    bfc1_ext = nc.declare_dram_parameter("bfc1c", [P, MH], f32, isOutput=False)
    out_ext = nc.declare_dram_parameter("out", [C, HH, WW], f32, isOutput=True)
    dbg = {}
    if _DEBUG:
        for nm, shape, dt in [
            ("dbg_q8", [P, 2, HH * (WW + 2) + 2], fp8),
            ("dbg_kv0", [P, TT // 2, 2 * C], fp8),
            ("dbg_kv1", [P, TT // 2, 2 * C], fp8),
            ("dbg_ctxacc", [P, 2 * P], f32),
            ("dbg_g0ps", [P, 2 * P], f32),
            ("dbg_dwps", [P, 4 * (WW + 2)], f32),
            ("dbg_ctxT", [P, 2 * P], mybir.dt.bfloat16),
            ("dbg_mcT", [P, 2, 9, 2 * P], fp8),
            ("dbg_dwc8", [P, 2, N], fp8),
            ("dbg_vsb", [P, N], mybir.dt.bfloat16),
            ("dbg_t18", [P, 2, N], fp8),
        ]:
            dbg[nm] = nc.declare_dram_parameter(nm, shape, dt, isOutput=True)

    with tile.TileContext(nc) as tc:
        with (
            tc.tile_pool(name="persist", bufs=1) as persist,
            tc.tile_pool(name="small", bufs=1) as small,
            tc.tile_pool(name="xbpool", bufs=2) as xbpool,
            tc.tile_pool(name="dtpool", bufs=2) as dtpool,
            tc.tile_pool(name="kvpool", bufs=2) as kvpool,
            tc.tile_pool(name="upool", bufs=1) as upool,
            tc.tile_pool(name="hpool", bufs=2) as hpool,
            tc.tile_pool(name="outsb", bufs=2) as outsb_pool,
            tc.tile_pool(name="psFc1", bufs=2, space="PSUM") as psFc1,
            tc.tile_pool(name="psHold", bufs=2, space="PSUM") as psHold,
            tc.tile_pool(name="psX", bufs=2, space="PSUM") as psX,
        ):
            # ---- constants built once --------------------------------
            ident = small.tile([P, P], bf16, tag="ident", name="ident")
            make_identity(nc, ident[:])
            ones8 = small.tile([1, 2, P], fp8, tag="ones8", name="ones8")
            nc.vector.memset(ones8[:], 1.0)
            thr = small.tile([P, 2 * C], bf16, tag="thr", name="thr")
            nc.vector.memset(thr[:, 0:C], 0.0)
            nc.vector.memset(thr[:, C:2 * C], -1e30)
            # double-buffered Mc (gen = rep % 2); zero halves stay zero
            mcT = [persist.tile([P, 2, 9, 2 * P], fp8, tag=f"mcT{g}",
                                name="mcT") for g in range(2)]
            for g in range(2):
                nc.vector.memset(mcT[g][:], 0.0)
            # q8 with 66-col padded rows (zero pads at col 0 and 65) so dw
            # taps are flat 1-D offset slices; memset once, drains only
            # touch data columns
            WP = WW + 2
            q8g = [persist.tile([P, 2, HH * WP + 2], fp8, tag=f"q8_{g}",
                                name="q8") for g in range(2)]
            for g in range(2):
                nc.gpsimd.memset(q8g[g][:], 0.0)
            ctxacc = [small.tile([P, 2 * P], f32, tag=f"ctxacc{g}",
                                 name="ctxacc") for g in range(2)]
            # block-diagonal at 32x32 head level: zero once, only diagonal
            # head blocks rewritten per rep
            ctxT_sb = [small.tile([P, 2 * P], bf16, tag=f"ctxT{g}",
                                  name="ctxT") for g in range(2)]
            for g in range(2):
                nc.vector.memset(ctxT_sb[g][:], 0.0)

            pcst = [small.tile([P, 8], f32, tag=f"pcst_{m}", name="pcst")
                    for m in range(2)]
            bq32 = [pcst[m][:, 0:1] for m in range(2)]
            A1A2s = [pcst[m][:, 1:2] for m in range(2)]
            u12b = [pcst[m][:, 2:3] for m in range(2)]
            A2_32 = [pcst[m][:, 3:4] for m in range(2)]
            invA2 = [pcst[m][:, 4:5] for m in range(2)]
            negB2oA2 = [pcst[m][:, 5:6] for m in range(2)]
            A1A2f = [pcst[m][:, 6:7] for m in range(2)]
            bkv_sb = small.tile([1, 2, 2 * C], fp8, tag="bkv", name="bkv")
            bfc1_sb = small.tile([P, MH], f32, tag="bfc1_sb", name="bfc1_sb")
            dwwt = [small.tile([P, 9], f32, tag=f"dww_{m}", name="dwwt")
                    for m in range(2)]
            diag_c = [small.tile([P, 9, P], bf16, tag=f"diagc_{m}",
                                 name="diagc") for m in range(2)]
            wqkv = persist.tile([P, 2, 3 * C], fp8, tag="wqkv", name="wqkv")
            wproj = persist.tile([P, 2, C], fp8, tag="wproj", name="wproj")
            wfc1 = persist.tile([P, 2, CM], fp8, tag="wfc1", name="wfc1")
            wfc2 = persist.tile([P, MH, C], fp8, tag="wfc2", name="wfc2")
            xq = persist.tile([P, 2, N], fp8, tag="xq", name="xq")
            # u12 as per-chunk tiles so early consumers don't wait on the
            # last chunk's write (coarse tile-level dependencies)
            u12 = [[[upool.tile([P, N // 4], bf16, tag=f"u12_{m}_{g}_{j}",
                                name="u12")
                     for j in range(4)] for m in range(2)] for g in range(2)]
            v_sb = [upool.tile([P, N], bf16, tag=f"vsb_{m}", name="v_sb")
                    for m in range(2)]

            x2d = x_ext[:].rearrange("c h w -> c (h w)")
            out2d = out_ext[:].rearrange("c h w -> c (h w)")

            # ---- one-time weight/constant loads --------------------------
            dma_engs = [nc.sync, nc.scalar, nc.sync, nc.scalar]
            for m in range(2):
                nc.gpsimd.dma_start(
                    out=pcst[m][:], in_=pcst_ext[m * P:(m + 1) * P, :]
                )
            nc.gpsimd.dma_start(out=bkv_sb[:], in_=bkv_ext[:].rearrange(
                "k n -> (k n)").rearrange("(o k n) -> o k n", o=1, k=2))
            nc.gpsimd.dma_start(out=bfc1_sb[:], in_=bfc1_ext[:])
            nc.sync.dma_start(
                out=wqkv[:], in_=wqkv_ext[:].rearrange("k p f -> p k f")
            )
            nc.gpsimd.dma_start(
                out=wproj[:], in_=wproj_ext[:].rearrange("k p c -> p k c")
            )
            nc.scalar.dma_start(
                out=wfc1[:], in_=wfc1_ext[:].rearrange("k p f -> p k f")
            )
            for k in range(4):
                dma_engs[k % 2].dma_start(
                    out=wfc2[:, 4 * k:4 * (k + 1), :],
                    in_=wfc2_ext[:].rearrange("k p c -> p k c")[
                        :, 4 * k:4 * (k + 1), :
                    ],
                )
            for m in range(2):
                nc.gpsimd.dma_start(
                    out=dwwt[m][:], in_=dww_ext[m * P:(m + 1) * P, :]
                )
                for i in range(9):
                    nc.gpsimd.tensor_scalar_mul(
                        diag_c[m][:, i, :], ident[:], dwwt[m][:, i:i + 1]
                    )

            def emit_loads(r):
                st = {}
                st["xb"] = [xbpool.tile([P, N], bf16, tag="xbpool", name="xb")
                            for _ in range(2)]
                xq_r = xq_ext[:].rearrange("k p n -> p k n")
                for j in range(8):
                    s = slice(j * 512, (j + 1) * 512)
                    dma_engs[j % 4].dma_start(out=xq[:, :, s],
                                              in_=xq_r[:, :, s])
                for j in range(4):
                    s = slice(j * 1024, (j + 1) * 1024)
                    for m in range(2):
                        rows = slice(m * P, (m + 1) * P)
                        dma_engs[(2 * j + m) % 4].dma_start(
                            out=st["xb"][m][:, s], in_=x2d[rows, s]
                        )
                st["kv8"] = [kvpool.tile([P, TT // 2, 2 * C], fp8,
                                         tag="kvpool", name="kv8")
                             for _ in range(2)]
                st["q8"] = q8g[r % 2]
                return st

            # ---------------- phase-1 pieces (rep r) ----------------------
            def kv_step(st, tt):
                ti, j = divmod(tt, TT // 2)
                tcols = slice(tt * P, (tt + 1) * P)
                ps = psX.tile([P, 2 * C], f32, tag="psX", name="kv_ps")
                nc.tensor.matmul(
                    ps[:], xq[:, :, tcols], wqkv[:, :, C:3 * C],
                    start=True, stop=False, perf_mode=DR,
                    skip_group_check=True,
                )
                nc.tensor.matmul(
                    ps[:], ones8[:], bkv_sb[:],
                    start=False, stop=True, perf_mode=DR,
                    skip_group_check=True,
                )
                # k = relu(32(k+bk)), v = 32(v+bv): max with {0 | -big}
                nc.vector.scalar_tensor_tensor(
                    st["kv8"][ti][:, j, :], ps[:], 1.0, thr[:], mult, amax,
                )

            def q_step(st, nt):
                cols = slice(nt * NTC, (nt + 1) * NTC)
                for m in range(2):
                    ps = psX.tile([P, NTC], f32, tag="psX", name="q_ps")
                    nc.tensor.matmul(
                        ps[:],
                        wqkv[:, :, m * P:(m + 1) * P],
                        xq[:, :, cols],
                        start=True, stop=True, perf_mode=DR,
                    )
                    q8v = st["q8"][:, m, 1:1 + HH * WP].rearrange(
                        "p (y x) -> p y x", x=WP)
                    nc.vector.tensor_scalar(
                        q8v[:, nt * YB:(nt + 1) * YB, 1:WW + 1],
                        ps[:].rearrange("p (y x) -> p y x", x=WW),
                        bq32[m], 0.0, add, amax,
                    )

            def ctx_group(st, g, gen):
                # pairs 4g..4g+3 (token tiles 8g..8g+7) -> psum -> ctxacc
                ps = psX.tile([P, 2 * P], f32, tag="psX", name="ctxg_ps")
                for pr in range(4):
                    pair = 4 * g + pr
                    ti, j = divmod(2 * pair, TT // 2)
                    for m in range(2):
                        # one start per psum bank: start marks the whole
                        # 2KB zero-region, so only the first mm may start
                        nc.tensor.matmul(
                            ps[:, m * P:(m + 1) * P],
                            st["kv8"][ti][:, j:j + 2,
                                          C + m * P:C + (m + 1) * P],
                            st["kv8"][ti][:, j:j + 2, m * P:(m + 1) * P],
                            start=(pr == 0 and m == 0),
                            stop=(pr == 3 and m == 1),
                            perf_mode=DR,
                            skip_group_check=True,
                        )
                if g == 0:
                    nc.vector.tensor_scalar(
                        ctxacc[gen][:], ps[:], SCALE / (SW * SW), None, mult,
                    )
                    if _DEBUG and st["rep"] == 0:
                        sb = small.tile([P, 2 * P], f32, tag="dbgg0",
                                        name="dbgg0")
                        nc.vector.tensor_copy(sb[:], ps[:])
                        nc.sync.dma_start(out=dbg["dbg_g0ps"][:], in_=sb[:])
                else:
                    nc.vector.scalar_tensor_tensor(
                        ctxacc[gen][:], ps[:], SCALE / (SW * SW),
                        ctxacc[gen][:], mult, add,
                    )

            def mc_build(gen):
                # extract only the within-head 32x32 diagonal blocks
                for h in range(NH):
                    m, r = divmod(h, 4)
                    rows = slice(HD * r, HD * r + HD)
                    cols = slice(m * P + HD * r, m * P + HD * r + HD)
                    nc.vector.tensor_copy(ctxT_sb[gen][rows, cols],
                                          ctxacc[gen][rows, cols])
                for m in range(2):
                    for g0, g1 in ((0, 4), (4, 8), (8, 9)):
                        ng = g1 - g0
                        ps = psX.tile([P, ng * P], f32, tag="psX",
                                      name="mc_ps")
                        for i in range(g0, g1):
                            nc.tensor.matmul(
                                ps[:, (i - g0) * P:(i - g0 + 1) * P],
                                ctxT_sb[gen][:, m * P:(m + 1) * P],
                                diag_c[m][:, i, :],
                                start=(i == g0), stop=(i == g1 - 1),
                                skip_group_check=True,
                            )
                        nc.scalar.activation(
                            mcT[gen][:, m, g0:g1, m * P:(m + 1) * P],
                            ps[:].rearrange("p (g q) -> p g q", g=ng),
                            Copy, bias=0.0, scale=BETA,
                        )

            # ---------------- steady pieces (rep s) -----------------------
            def clip(dy, dx, y0):
                ys = max(y0, -dy)
                ye = min(y0 + YB, HH - dy)
                xs = max(0, -dx)
                xe = min(WW, WW - dx)
                return ys, ye, xs, xe

            def u12_piece(st, gen, j):
                # eager u12 (rep r) chunk j of 4, both m chunks; bf16
                # in/out SBUF on DVE hits the 2x path
                s = slice(j * 1024, (j + 1) * 1024)
                for m in range(2):
                    nc.vector.tensor_scalar(
                        u12[gen][m][j][:], st["xb"][m][:, s],
                        A1A2f[m], u12b[m], mult, add,
                    )

            def dw_phase(st, gen, nt, mo):
                # two 4-row halves; taps are flat 1-D offset slices of the
                # 66-col padded q8 (out pad cols compute junk, never read)
                q8 = st["q8"]
                for rh in range(2):
                    r0 = nt * YB + rh * 4
                    ps = psX.tile([P, 4 * WP], f32, tag="psX", name="dw_ps")
                    for i, (dy, dx) in enumerate(TAPS):
                        ysr = max(r0, -dy)
                        yer = min(r0 + 4, HH - dy)
                        ti = (dy + 1) * 3 + (dx + 1)
                        nc.tensor.matmul(
                            ps[:, (ysr - r0) * WP:(yer - r0) * WP],
                            mcT[gen][:, :, ti, mo * P:(mo + 1) * P],
                            q8[:, :, 1 + (ysr + dy) * WP + dx:
                               1 + (yer + dy) * WP + dx],
                            start=(i == 0), stop=(i == 8),
                            perf_mode=DR,
                            skip_group_check=True,
                        )
                    if _DEBUG and st["rep"] == 0 and nt == 0 and mo == 0 \
                            and rh == 0:
                        sb = small.tile([P, 4 * WP], f32, tag="dbgdw",
                                        name="dbgdw")
                        nc.vector.tensor_copy(sb[:], ps[:])
                        nc.sync.dma_start(out=dbg["dbg_dwps"][:], in_=sb[:])
                    # dwc8 = 16*dwconv = ps/32 (dw_b folded into B1')
                    nc.vector.tensor_scalar(
                        st["dwc8"][:, mo, r0 * WW:(r0 + 4) * WW].rearrange(
                            "p (y x) -> p y x", x=WW),
                        ps[:].rearrange("p (y x) -> p y x", x=WP)[
                            :, :, 1:WW + 1],
                        1.0 / SW, None, mult,
                    )

            def proj_phase(st, gen, nt, mo, t1_eng=None):
                cols = slice(nt * NTC, (nt + 1) * NTC)
                ps = psX.tile([P, NTC], f32, tag="psX", name="proj_ps")
                nc.tensor.matmul(
                    ps[:],
                    wproj[:, :, mo * P:(mo + 1) * P],
                    st["dwc8"][:, :, cols],
                    start=True, stop=True, perf_mode=DR,
                )
                nc.vector.scalar_tensor_tensor(
                    v_sb[mo][:, cols], ps[:], A1A2s[mo],
                    u12[gen][mo][nt // 2][:, (nt % 2) * NTC:
                                          (nt % 2 + 1) * NTC],
                    mult, add,
                )
                (t1_eng or nc.gpsimd).tensor_scalar(
                    st["t1_8"][:, mo, cols], v_sb[mo][:, cols],
                    invA2[mo], negB2oA2[mo], mult, add,
                )

            def fc1_piece(st, p, k0, k1):
                n0 = 2 * p
                colsA = slice(n0 * NTC, (n0 + 1) * NTC)
                colsB = slice((n0 + 1) * NTC, (n0 + 2) * NTC)
                if k0 == 0:
                    st["h_sb"] = hpool.tile([P, MH, 2 * NTC], fp8,
                                            tag="hpool", name="h_sb")
                for kt in range(k0, k1):
                    ps = psFc1.tile([P, 2, NTC], f32, tag="psFc1",
                                    name="fc1_ps")
                    for half, cols in ((0, colsA), (1, colsB)):
                        nc.tensor.matmul(
                            ps[:, half, :],
                            wfc1[:, :, kt * P:(kt + 1) * P],
                            st["t1_8"][:, :, cols],
                            start=True, stop=True, perf_mode=DR,
                            skip_group_check=True,
                        )
                    nc.scalar.activation(
                        st["h_sb"][:, kt, :],
                        ps[:].rearrange("p h q -> p (h q)"),
                        Gelu, bias=bfc1_sb[:, kt:kt + 1], scale=1.0 / SW,
                    )

            def fc2_piece(st, p, half, ktp0, ktp1, fc2_ps):
                for ktp in range(ktp0, ktp1):
                    kt = 2 * ktp
                    for mo in range(2):
                        nc.tensor.matmul(
                            fc2_ps[mo][:],
                            wfc2[:, kt:kt + 2, mo * P:(mo + 1) * P],
                            st["h_sb"][:, kt:kt + 2,
                                       half * NTC:(half + 1) * NTC],
                            start=(ktp == 0),
                            stop=(ktp == MH // 2 - 1),
                            perf_mode=DR,
                            skip_group_check=True,
                        )

            def out_phase(nt, fc2_ps):
                cols = slice(nt * NTC, (nt + 1) * NTC)
                for mo in range(2):
                    ot = outsb_pool.tile([P, NTC], f32, tag="outsb",
                                         name="outsb")
                    nc.vector.scalar_tensor_tensor(
                        ot[:], fc2_ps[mo][:], A2_32[mo],
                        v_sb[mo][:, cols], mult, add,
                    )
                    nc.sync.dma_start(
                        out=out2d[mo * P:(mo + 1) * P, cols], in_=ot[:],
                    )

            # ---------------- piece lists + interleave --------------------
            def phase1_pieces(st, gen):
                ps_list = []
                for tt in range(TT):
                    ps_list.append(lambda tt=tt: kv_step(st, tt))
                    if tt % 4 == 3:
                        ps_list.append(lambda nt=tt // 4: q_step(st, nt))
                    if tt % 8 == 1 and tt > 1:
                        ps_list.append(
                            lambda j=(tt - 9) // 8: u12_piece(st, gen, j))
                    if tt % 8 == 7 and tt >= 15:
                        g = (tt - 15) // 8  # groups 0..2 after tiles 15,23,31
                        ps_list.append(lambda g=g: ctx_group(st, g, gen))
                ps_list.append(lambda: u12_piece(st, gen, 3))
                ps_list.append(lambda: ctx_group(st, 3, gen))
                ps_list.append(lambda: mc_build(gen))
                return ps_list

            def get_fc2(st, key):
                st["fc2_tiles"][key] = [psHold.tile([P, NTC], f32,
                                                    tag="psHold",
                                                    name="fc2_ps")
                                        for _ in range(2)]
                return st["fc2_tiles"][key]

            def steady_head_pieces(st, gen):
                # pipeline-fill of rep r's steady loop, emitted at the tail
                # of body r (overlaps rep r-1's drain)
                st["dwc8"] = dtpool.tile([P, 2, N], fp8, tag="dtpool",
                                         name="dwc8")
                st["t1_8"] = dtpool.tile([P, 2, N], fp8, tag="dtpool",
                                         name="t1_8")
                st["fc2_tiles"] = {}

                def pre(nt):
                    # head t1_8 on DVE: shortens the boundary fill chain
                    dw_phase(st, gen, nt, 0)
                    dw_phase(st, gen, nt, 1)
                    proj_phase(st, gen, nt, 0, t1_eng=nc.vector)
                    proj_phase(st, gen, nt, 1, t1_eng=nc.vector)

                def h1():
                    get_fc2(st, (0, 0))
                    fc1_piece(st, 0, 0, 4)

                return [lambda: pre(0), lambda: pre(1), h1]

            def steady_rest_pieces(st, gen):
                ps_list = []
                fc2_tiles = st["fc2_tiles"]
                for p in range(NT // 2):
                    def s1(p=p):
                        if p > 0:
                            get_fc2(st, (p, 0))
                            fc1_piece(st, p, 0, 4)
                    def s2(p=p):
                        if 2 * p + 2 < NT:
                            dw_phase(st, gen, 2 * p + 2, 0)
                            dw_phase(st, gen, 2 * p + 2, 1)
                    def s3(p=p):
                        fc1_piece(st, p, 4, 8)
                        fc2_piece(st, p, 0, 0, 4, fc2_tiles[(p, 0)])
                    def s4(p=p):
                        if 2 * p + 2 < NT:
                            proj_phase(st, gen, 2 * p + 2, 0)
                            proj_phase(st, gen, 2 * p + 2, 1)
                    def s5(p=p):
                        fc1_piece(st, p, 8, 12)
                        fc2_piece(st, p, 0, 4, 6, fc2_tiles[(p, 0)])
                    def s6(p=p):
                        if 2 * p + 3 < NT:
                            dw_phase(st, gen, 2 * p + 3, 0)
                            dw_phase(st, gen, 2 * p + 3, 1)
                    def s7(p=p):
                        fc1_piece(st, p, 12, 16)
                        fc2_piece(st, p, 0, 6, 8, fc2_tiles[(p, 0)])
                        out_phase(2 * p, fc2_tiles[(p, 0)])
                        get_fc2(st, (p, 1))
                        fc2_piece(st, p, 1, 0, 4, fc2_tiles[(p, 1)])
                    def s8(p=p):
                        if 2 * p + 3 < NT:
                            proj_phase(st, gen, 2 * p + 3, 0)
                            proj_phase(st, gen, 2 * p + 3, 1)
                    def s9(p=p):
                        fc2_piece(st, p, 1, 4, 8, fc2_tiles[(p, 1)])
                        out_phase(2 * p + 1, fc2_tiles[(p, 1)])
                    if p == 0:
                        ps_list += [s2, s3, s4, s5, s6, s7, s8, s9]
                    else:
                        ps_list += [s1, s2, s3, s4, s5, s6, s7, s8, s9]
                return ps_list

            def emit_debug(st, gen):
                nc.sync.dma_start(out=dbg["dbg_q8"][:], in_=st["q8"][:])
                nc.sync.dma_start(out=dbg["dbg_kv0"][:], in_=st["kv8"][0][:])
                nc.sync.dma_start(out=dbg["dbg_kv1"][:], in_=st["kv8"][1][:])
                nc.sync.dma_start(out=dbg["dbg_ctxacc"][:],
                                  in_=ctxacc[gen][:])
                nc.sync.dma_start(out=dbg["dbg_ctxT"][:], in_=ctxT_sb[gen][:])
                nc.sync.dma_start(out=dbg["dbg_mcT"][:], in_=mcT[gen][:])
                nc.sync.dma_start(out=dbg["dbg_dwc8"][:], in_=st["dwc8"][:])
                nc.sync.dma_start(out=dbg["dbg_vsb"][:], in_=v_sb[0][:])
                nc.sync.dma_start(out=dbg["dbg_t18"][:], in_=st["t1_8"][:])

            states = {}
            for body in range(reps + 1):
                p1 = []
                p2 = []
                if body < reps:
                    st = emit_loads(body)
                    st["rep"] = body
                    states[body] = st
                    p1 = (phase1_pieces(st, body % 2)
                          + steady_head_pieces(st, body % 2))
                if body >= 1:
                    sprev = states[body - 1]
                    p2 = steady_rest_pieces(sprev, (body - 1) % 2)
                # interleave with phase-1 (+ next steady head) finishing at
                # ~80% of the steady list, so the head's pipeline-fill chain
                # overlaps the previous rep's fc tail
                na, nb = len(p1), len(p2)
                ia = ib = 0
                while ia < na or ib < nb:
                    if ib < nb and (ia >= na or ia * nb * 10 >= ib * na * 6):
                        p2[ib]()
                        ib += 1
                    else:
                        p1[ia]()
                        ia += 1
                if body >= 1:
                    if _DEBUG and body == reps:
                        emit_debug(states[body - 1], (body - 1) % 2)
                    del states[body - 1]

    nc.compile()
    return nc


def _get_nc(reps=1):
    key = ("nc", reps)
    if key not in _CACHE:
        _CACHE[key] = _build_nc(reps)
    return _CACHE[key]


def _prep_shared(inputs):
    import ml_dtypes

    bf = ml_dtypes.bfloat16
    e4 = ml_dtypes.float8_e4m3
    f = lambda k: np.asarray(inputs[k], dtype=np.float32)

    rs1 = 1.0 / np.sqrt(f("bn1_v") + EPS)
    gr1 = f("bn1_g") * rs1
    A1 = gr1 + f("alpha1")
    # dw_b and bproj folded: proj_full = Wproj^T(dwconv + dw_b) + bproj
    B1 = (f("bn1_b") - f("bn1_m") * gr1
          + A1 * (f("bproj") + f("Wproj").T @ f("dw_b")))
    rs2 = 1.0 / np.sqrt(f("bn2_v") + EPS)
    gr2 = f("bn2_g") * rs2
    A2 = gr2 + f("alpha2")
    B2 = f("bn2_b") - f("bn2_m") * gr2 + A2 * f("bfc2")

    A1A2 = A1 * A2
    u12b = A2 * B1 + B2
    invA2 = 1.0 / A2
    negB2oA2 = -B2 / A2

    pcst = np.stack(
        [f("bqkv")[:C] * SW, A1A2 / (SD * SW), u12b, A2 / SW, invA2,
         negB2oA2, A1A2, np.zeros_like(A2)],
        axis=1,
    )
    bkv8 = np.zeros((2, 2 * C), np.float32)
    bkv8[0] = f("bqkv")[C:] * SW

    return {
        "wqkv8": np.ascontiguousarray(
            (f("Wqkv") * SW).astype(e4).reshape(2, P, 3 * C)),
        "wproj8": np.ascontiguousarray(
            (f("Wproj") * SW).astype(e4).reshape(2, P, C)),
        "wfc18": np.ascontiguousarray(
            (f("Wfc1") * SW).astype(e4).reshape(2, P, CM)),
        "wfc28": np.ascontiguousarray(
            (f("Wfc2") * SW).astype(e4).reshape(MH, P, C)),
        "dww": np.ascontiguousarray(f("dw_w").reshape(C, 9)),
        "pcst": np.ascontiguousarray(pcst.astype(np.float32)),
        "bkv8": np.ascontiguousarray(bkv8.astype(e4)),
        "bfc1c": np.ascontiguousarray(f("bfc1").reshape(MH, P).T),
    }


def _per_core_maps(inputs):
    import ml_dtypes

    shared = _prep_shared(inputs)
    xf = np.asarray(inputs["x"], dtype=np.float32).reshape(B, C, HH, WW)
    in_maps = []
    for i in range(B):
        x2d = xf[i].reshape(C, N)
        in_maps.append(dict(
            shared,
            x=np.ascontiguousarray(x2d.astype(ml_dtypes.bfloat16)
                                   ).reshape(C, HH, WW),
            xq=np.ascontiguousarray(
                x2d.astype(ml_dtypes.float8_e4m3).reshape(2, P, N)),
        ))
    return in_maps


def kernel(**inputs):
    from concourse.bass_utils import run_bass_kernel_spmd

    nc = _get_nc()
    in_maps = _per_core_maps(inputs)
    res = run_bass_kernel_spmd(nc, in_maps, core_ids=list(range(B)))
    return np.stack([res.results[i]["out"] for i in range(B)], axis=0)


def make_in_maps(inputs):
    return _per_core_maps(inputs)
